# revision 4
# baseline (speedup 1.0000x reference)
"""HD95 loss kernel for Trainium2 (Bass/Tile), 8 NeuronCores.

Reference semantics: per image, threshold pred/true at 0.5, compact nonzero
pixel indices in row-major order, split each point list into blocks of 1000,
and for every (point, opposite-side block) pair take the min Euclidean
distance; the HD95 is the 95th linear-interpolation quantile over all finite
such mins (both directions), averaged over the batch.

Device algorithm (per image & direction, "queries" vs "ref blocks"):
separable squared-EDT with the row stage precomputed on the host.

  host:     g[x, blk, c] = min_{a : pixel(b0+c, a) in blk} (x-a)^2
            (exact integer table, bf16-rounded; sentinel 2^26 for empty
            candidate rows), plus per-candidate row features
            rtop = [1, 1, -2b, b2h, b2l] with b = b0+c (bf16-exact split
            of b^2 into a multiple of 128 plus a <128 remainder).
  device:   min d^2(q, blk) = min_c ( (y_q - (b0+c))^2 + g[x_q, c] )
            one accumulating bf16 matmul per 128-query tile:
            [onehot(x_q); y2h, y2l, y, 1, 1] @ [g ; rtop]  (K=101 rows),
            then a DVE min-reduce over the 24 candidates of each block.
            The y-part is bit-exact; g carries <=2^-9 relative rounding,
            far inside the 2e-2 harness gate.

Core mapping: 8 cores = 4 (image x direction) jobs x 2 halves of 2432
query slots. Host does the O(N) compaction/feature build and the final
O(50k) quantile; device does all O(K x window) distance work. The input
lhsT DMA is split into 16 partition-slices so it spreads across the 16
HW DMA engines (a single dma_start lands on one engine at ~15 GB/s).
"""

import numpy as np

H = 96
W = 96
BLK = 1000        # reference cdist block size
NBLK = 5          # blocks per side (asserted from the data regime)
CAND = 24         # candidate image rows per block window (spans <= 23 here)
M = NBLK * CAND   # matmul free size (120 candidate columns)
NTILES = 19       # query tiles of 128 per core
QHALF = NTILES * 128  # 2432 query slots per core
GRP = 4           # tiles per PSUM bank (4*120 f32 = 1920B <= 2KB bank)
NGRP = (NTILES + GRP - 1) // GRP  # 5 groups (last has 3 tiles)
BIG = float(2 ** 26)  # sentinel (bf16-exact, >> max real d^2 of 18050)
NCORES = 8
NDMA = 8          # input-DMA column slices (one per HW DMA engine)

_CACHE = {}


def _build_nc():
    import concourse.bacc as bacc
    import concourse.mybir as mybir
    import concourse.tile as tile

    f32 = mybir.dt.float32
    bf16 = mybir.dt.bfloat16
    # Bacc (not raw Bass): its compile() runs move_matmul_waits_to_ldweights
    # + generate_event_semaphores, which legalize multi-wait instructions
    # (TRN2 allows at most one sync wait per instruction).
    nc = bacc.Bacc("TRN2", target_bir_lowering=False, debug=False)

    lhsT = nc.declare_dram_parameter("lhsT", [101, QHALF], bf16, isOutput=False)
    ghr = nc.declare_dram_parameter("ghr", [101, M], bf16, isOutput=False)
    mins = nc.declare_dram_parameter(
        "mins", [128, NTILES * NBLK], f32, isOutput=True
    )

    X = mybir.AxisListType.X
    MIN = mybir.AluOpType.min

    with tile.TileContext(nc) as tc:
        with (
            tc.tile_pool(name="const", bufs=1) as const,
            tc.tile_pool(name="ps", bufs=NGRP, space="PSUM") as psp,
        ):
            t_lhsT = const.tile([101, QHALF], bf16)
            t_ghr = const.tile([101, M], bf16)
            t_out = const.tile([128, NTILES * NBLK], f32)

            # rhs first (needed by every matmul), then the big lhsT in
            # column slices alternating between the two HWDGE queues
            # (Sync + Scalar) so the ~720ns per-dma_start issue cost
            # parallelizes and each slice lands on its own DMA engine.
            # Column slices let early query tiles start while later
            # slices are still in flight.
            nc.scalar.dma_start(t_ghr[:], ghr[:])
            bounds = np.linspace(0, QHALF, NDMA + 1).astype(int)
            for i in range(NDMA):
                sl = slice(int(bounds[i]), int(bounds[i + 1]))
                eng = nc.sync if i % 2 == 0 else nc.scalar
                eng.dma_start(t_lhsT[:, sl], lhsT[:, sl])

            for g in range(NGRP):
                nt = min(GRP, NTILES - g * GRP)
                ps = psp.tile([128, GRP, NBLK, CAND], f32, tag="ps")
                for k in range(nt):
                    t = g * GRP + k
                    nc.tensor.matmul(
                        ps[:, k, :, :],
                        t_lhsT[:, t * 128 : (t + 1) * 128],
                        t_ghr[:],
                        start=True,
                        stop=True,
                    )
                o0 = g * GRP * NBLK
                nc.vector.tensor_reduce(
                    t_out[:, o0 : o0 + nt * NBLK],
                    ps[:, 0:nt, :, :],
                    axis=X,
                    op=MIN,
                )
                nc.sync.dma_start(
                    mins[:, o0 : o0 + nt * NBLK], t_out[:, o0 : o0 + nt * NBLK]
                )

    nc.compile()
    return nc


def _get_nc():
    if "nc" not in _CACHE:
        _CACHE["nc"] = _build_nc()
    return _CACHE["nc"]


def _bf16(a):
    from ml_dtypes import bfloat16

    return np.asarray(a, np.float32).astype(bfloat16)


def _hilo(v):
    """Split integer-valued array into (multiple-of-128, remainder<128)."""
    v = np.asarray(v, np.float64)
    lo = np.mod(v, 128.0)
    return (v - lo).astype(np.float32), lo.astype(np.float32)


def _side_points(img):
    """Compacted nonzero pixel coords, row-major ascending (matches
    jnp.nonzero order)."""
    m = (np.asarray(img) > 0.5).reshape(-1)
    idx = np.nonzero(m)[0]
    ys = (idx // W).astype(np.int64)
    xs = (idx % W).astype(np.int64)
    return ys, xs


def _feat5_queries(vals):
    """[v2h, v2l, v, 1, 1] feature rows for the squared-term side."""
    v = np.asarray(vals, np.float64)
    h, l = _hilo(v * v)
    one = np.ones_like(v, np.float32)
    return np.stack([h, l, v.astype(np.float32), one, one])


def _feat5_refs(vals):
    """[1, 1, -2v, v2h, v2l] feature rows for the reference side."""
    v = np.asarray(vals, np.float64)
    h, l = _hilo(v * v)
    one = np.ones_like(v, np.float32)
    return np.stack([one, one, (-2.0 * v).astype(np.float32), h, l])


def _build_core_inputs(q_ys, q_xs, r_ys, r_xs):
    """Host-side feature build for one (image, direction) job.

    q_*: query points (cnt_q), r_*: reference points (cnt_r, split into
    NBLK blocks of BLK in compacted order). Returns two per-core input
    maps, or None if the data falls outside the compiled regime.
    """
    cnt_q, cnt_r = len(q_ys), len(r_ys)
    if not (0 < cnt_q <= 2 * QHALF and 0 < cnt_r <= NBLK * BLK):
        return None
    if (cnt_r + BLK - 1) // BLK != NBLK:
        return None

    xgrid = np.arange(W, dtype=np.float64)
    g = np.full((W, NBLK, CAND), BIG, np.float32)  # rows 0..95 of rhs
    rtop = np.empty((5, NBLK, CAND), np.float32)
    for blk in range(NBLK):
        lo, hi = blk * BLK, min((blk + 1) * BLK, cnt_r)
        ys_b, xs_b = r_ys[lo:hi], r_xs[lo:hi]
        b0 = int(ys_b[0])
        if int(ys_b[-1]) - b0 + 1 > CAND:
            return None
        for c in np.unique(ys_b - b0):
            xs_c = xs_b[ys_b - b0 == c].astype(np.float64)
            d = np.abs(xgrid[:, None] - xs_c[None, :]).min(1)
            g[:, blk, c] = (d * d).astype(np.float32)
        rtop[:, blk, :] = _feat5_refs(b0 + np.arange(CAND))
    ghr = _bf16(
        np.concatenate([g.reshape(W, -1), rtop.reshape(5, -1)], axis=0)
    )

    # lhsT rows: 0..95 onehot(x), 96..100 yfeat; padded slots zero
    s2_lhsT = np.zeros((101, 2 * QHALF), np.float32)
    s2_lhsT[q_xs, np.arange(cnt_q)] = 1.0
    s2_lhsT[96:101, :cnt_q] = _feat5_queries(q_ys)

    maps = []
    for half in range(2):
        hs = slice(half * QHALF, (half + 1) * QHALF)
        maps.append({"lhsT": _bf16(s2_lhsT[:, hs]), "ghr": ghr})
    return maps


def _quantile95(vals):
    """torch.quantile / jnp.nanquantile 'linear' on finite values."""
    v = np.sort(np.asarray(vals, np.float64))
    n = v.size
    pos = 0.95 * (n - 1)
    lo = int(np.floor(pos))
    hi = min(lo + 1, n - 1)
    frac = pos - lo
    return v[lo] * (1.0 - frac) + v[hi] * frac


def _hd95_numpy_fallback(pred, true):
    """Pure-numpy path for data outside the compiled regime."""
    p_ys, p_xs = _side_points(pred)
    t_ys, t_xs = _side_points(true)
    if len(p_ys) == 0 or len(t_ys) == 0:
        return None
    pc = np.stack([p_ys, p_xs], -1).astype(np.float32)
    tc = np.stack([t_ys, t_xs], -1).astype(np.float32)
    vals = []
    for qc, rc in ((pc, tc), (tc, pc)):
        nbr = (len(rc) + BLK - 1) // BLK
        for jb in range(nbr):
            b = rc[jb * BLK : (jb + 1) * BLK]
            d2 = (
                (qc * qc).sum(-1)[:, None]
                + (b * b).sum(-1)[None, :]
                - 2.0 * (qc @ b.T)
            )
            vals.append(np.sqrt(np.maximum(d2.min(1), 0.0).astype(np.float32)))
    return _quantile95(np.concatenate(vals))


def _run_device(in_maps, trace=False):
    from concourse.bass_utils import run_bass_kernel_spmd

    nc = _get_nc()
    return run_bass_kernel_spmd(nc, in_maps, list(range(NCORES)), trace=trace)


def _decode_mins(raw):
    """[128, NTILES*NBLK] device layout -> [QHALF, NBLK] query-major d^2."""
    # column g*GRP*NBLK + k*NBLK + blk holds tile t = g*GRP+k; query
    # q = t*128 + partition
    return (
        raw.reshape(128, NTILES, NBLK).transpose(1, 0, 2).reshape(QHALF, NBLK)
    )


def kernel(input, target, _trace=False, _results_out=None):
    input = np.asarray(input)
    target = np.asarray(target)
    nimg = input.shape[0]

    # jobs: (image, direction). dir 0: queries=pred, refs=true (row mins);
    # dir 1: queries=true, refs=pred (col mins).
    jobs = []
    in_maps = []
    fallback = {}
    ok_mask = []
    for i in range(nimg):
        p_ys, p_xs = _side_points(input[i])
        t_ys, t_xs = _side_points(target[i])
        ok = len(p_ys) > 0 and len(t_ys) > 0
        ok_mask.append(ok)
        if not ok:
            continue
        built_row = _build_core_inputs(p_ys, p_xs, t_ys, t_xs)
        built_col = _build_core_inputs(t_ys, t_xs, p_ys, p_xs)
        if built_row is None or built_col is None or nimg != 2:
            fallback[i] = _hd95_numpy_fallback(input[i], target[i])
            continue
        jobs.append((i, 0, len(p_ys)))
        in_maps.extend(built_row)
        jobs.append((i, 1, len(t_ys)))
        in_maps.extend(built_col)

    hds = {}
    if jobs:
        while len(in_maps) < NCORES:  # pad to the full 8-core SPMD launch
            in_maps.append({k: v.copy() for k, v in in_maps[0].items()})
        res = _run_device(in_maps[:NCORES], trace=_trace)
        if _results_out is not None:
            _results_out.append(res)
        per_img_vals = {}
        for j, (img, _dir, cnt_q) in enumerate(jobs):
            d2 = np.concatenate(
                [
                    _decode_mins(res.results[2 * j]["mins"]),
                    _decode_mins(res.results[2 * j + 1]["mins"]),
                ]
            )[:cnt_q]
            assert d2.max() < 2.0 ** 25, "sentinel leaked into mins"
            dist = np.sqrt(d2.astype(np.float32))
            per_img_vals.setdefault(img, []).append(dist.ravel())
        for img, chunks in per_img_vals.items():
            hds[img] = _quantile95(np.concatenate(chunks))
    hds.update(fallback)

    n_ok = sum(ok_mask)
    if n_ok == 0:
        return np.float32(np.inf)
    total = sum(hds[i] for i in range(nimg) if ok_mask[i])
    return np.float32(total / n_ok)


# revision 5
# speedup vs baseline: 1.7838x; 1.7838x over previous
"""HD95 loss kernel for Trainium2 (Bass/Tile), 8 NeuronCores.

Reference semantics: per image, threshold pred/true at 0.5, compact nonzero
pixel indices in row-major order, split each point list into blocks of 1000,
and for every (point, opposite-side block) pair take the min Euclidean
distance; the HD95 is the 95th linear-interpolation quantile over all finite
such mins (both directions), averaged over the batch.

Device algorithm (per image & direction, "queries" vs "ref blocks"):
separable squared-EDT with the row stage precomputed on the host.

  host:     g[x, blk, c] = min_{a : pixel(b0+c, a) in blk} (x-a)^2
            (exact integer table, bf16-rounded; sentinel 2^26 for empty
            candidate rows), plus per-candidate row features
            rtop = [1, 1, -2b, b2h, b2l] with b = b0+c (bf16-exact split
            of b^2 into a multiple of 128 plus a <128 remainder).
  device:   min d^2(q, blk) = min_c ( (y_q - (b0+c))^2 + g[x_q, c] )
            one accumulating bf16 matmul per 128-query tile:
            [onehot(x_q); y2h, y2l, y, 1, 1] @ [g ; rtop]  (K=101 rows),
            then a DVE min-reduce over the 24 candidates of each block.
            The y-part is bit-exact; g carries <=2^-9 relative rounding,
            far inside the 2e-2 harness gate.

Core mapping: 8 cores = 4 (image x direction) jobs x 2 halves of 2432
query slots. Host does the O(N) compaction/feature build and the final
O(50k) quantile; device does all O(K x window) distance work. The input
lhsT DMA is split into 16 partition-slices so it spreads across the 16
HW DMA engines (a single dma_start lands on one engine at ~15 GB/s).
"""

import numpy as np

H = 96
W = 96
BLK = 1000        # reference cdist block size
NBLK = 5          # blocks per side (asserted from the data regime)
CAND = 24         # candidate image rows per block window (spans <= 23 here)
M = NBLK * CAND   # matmul free size (120 candidate columns)
NTILES = 19       # query tiles of 128 per core
QHALF = NTILES * 128  # 2432 query slots per core
GRP = 4           # tiles per PSUM bank (4*120 f32 = 1920B <= 2KB bank)
NGRP = (NTILES + GRP - 1) // GRP  # 5 groups (last has 3 tiles)
BIG = float(2 ** 26)  # sentinel (bf16-exact, >> max real d^2 of 18050)
NCORES = 8
NDMA = 8          # input-DMA column slices (one per HW DMA engine)

_CACHE = {}


def _build_nc():
    import concourse.bacc as bacc
    import concourse.mybir as mybir
    import concourse.tile as tile

    f32 = mybir.dt.float32
    bf16 = mybir.dt.bfloat16
    # Bacc (not raw Bass): its compile() runs move_matmul_waits_to_ldweights
    # + generate_event_semaphores, which legalize multi-wait instructions
    # (TRN2 allows at most one sync wait per instruction).
    nc = bacc.Bacc("TRN2", target_bir_lowering=False, debug=False)

    lhsT = nc.declare_dram_parameter("lhsT", [101, QHALF], bf16, isOutput=False)
    ghr = nc.declare_dram_parameter("ghr", [101, M], bf16, isOutput=False)
    mins = nc.declare_dram_parameter(
        "mins", [128, NTILES * NBLK], f32, isOutput=True
    )

    X = mybir.AxisListType.X
    MIN = mybir.AluOpType.min

    with tile.TileContext(nc) as tc:
        with (
            tc.tile_pool(name="const", bufs=1) as const,
            tc.tile_pool(name="ps", bufs=NGRP, space="PSUM") as psp,
        ):
            t_lhsT = const.tile([101, QHALF], bf16)
            t_ghr = const.tile([101, M], bf16)
            t_out = const.tile([128, NTILES * NBLK], f32)

            # rhs first (needed by every matmul), then the big lhsT in
            # partition-row slices (full 4864B lines keep per-packet DMA
            # efficiency) alternating between the two HWDGE queues
            # (Sync + Scalar): the ~720ns per-dma_start issue cost
            # parallelizes across queues and each slice lands on its own
            # DMA engine (~15 GB/s each).
            nc.scalar.dma_start(t_ghr[:], ghr[:])
            bounds = np.linspace(0, 101, NDMA + 1).astype(int)
            for i in range(NDMA):
                sl = slice(int(bounds[i]), int(bounds[i + 1]))
                eng = nc.sync if i % 2 == 0 else nc.scalar
                eng.dma_start(t_lhsT[sl, :], lhsT[sl, :])

            for g in range(NGRP):
                nt = min(GRP, NTILES - g * GRP)
                ps = psp.tile([128, GRP, NBLK, CAND], f32, tag="ps")
                for k in range(nt):
                    t = g * GRP + k
                    nc.tensor.matmul(
                        ps[:, k, :, :],
                        t_lhsT[:, t * 128 : (t + 1) * 128],
                        t_ghr[:],
                        start=True,
                        stop=True,
                    )
                o0 = g * GRP * NBLK
                nc.vector.tensor_reduce(
                    t_out[:, o0 : o0 + nt * NBLK],
                    ps[:, 0:nt, :, :],
                    axis=X,
                    op=MIN,
                )
                nc.sync.dma_start(
                    mins[:, o0 : o0 + nt * NBLK], t_out[:, o0 : o0 + nt * NBLK]
                )

    nc.compile()
    return nc


def _get_nc():
    if "nc" not in _CACHE:
        _CACHE["nc"] = _build_nc()
    return _CACHE["nc"]


def _bf16(a):
    from ml_dtypes import bfloat16

    return np.asarray(a, np.float32).astype(bfloat16)


def _hilo(v):
    """Split integer-valued array into (multiple-of-128, remainder<128)."""
    v = np.asarray(v, np.float64)
    lo = np.mod(v, 128.0)
    return (v - lo).astype(np.float32), lo.astype(np.float32)


def _side_points(img):
    """Compacted nonzero pixel coords, row-major ascending (matches
    jnp.nonzero order)."""
    m = (np.asarray(img) > 0.5).reshape(-1)
    idx = np.nonzero(m)[0]
    ys = (idx // W).astype(np.int64)
    xs = (idx % W).astype(np.int64)
    return ys, xs


def _feat5_queries(vals):
    """[v2h, v2l, v, 1, 1] feature rows for the squared-term side."""
    v = np.asarray(vals, np.float64)
    h, l = _hilo(v * v)
    one = np.ones_like(v, np.float32)
    return np.stack([h, l, v.astype(np.float32), one, one])


def _feat5_refs(vals):
    """[1, 1, -2v, v2h, v2l] feature rows for the reference side."""
    v = np.asarray(vals, np.float64)
    h, l = _hilo(v * v)
    one = np.ones_like(v, np.float32)
    return np.stack([one, one, (-2.0 * v).astype(np.float32), h, l])


def _build_core_inputs(q_ys, q_xs, r_ys, r_xs):
    """Host-side feature build for one (image, direction) job.

    q_*: query points (cnt_q), r_*: reference points (cnt_r, split into
    NBLK blocks of BLK in compacted order). Returns two per-core input
    maps, or None if the data falls outside the compiled regime.
    """
    cnt_q, cnt_r = len(q_ys), len(r_ys)
    if not (0 < cnt_q <= 2 * QHALF and 0 < cnt_r <= NBLK * BLK):
        return None
    if (cnt_r + BLK - 1) // BLK != NBLK:
        return None

    xgrid = np.arange(W, dtype=np.float64)
    g = np.full((W, NBLK, CAND), BIG, np.float32)  # rows 0..95 of rhs
    rtop = np.empty((5, NBLK, CAND), np.float32)
    for blk in range(NBLK):
        lo, hi = blk * BLK, min((blk + 1) * BLK, cnt_r)
        ys_b, xs_b = r_ys[lo:hi], r_xs[lo:hi]
        b0 = int(ys_b[0])
        if int(ys_b[-1]) - b0 + 1 > CAND:
            return None
        for c in np.unique(ys_b - b0):
            xs_c = xs_b[ys_b - b0 == c].astype(np.float64)
            d = np.abs(xgrid[:, None] - xs_c[None, :]).min(1)
            g[:, blk, c] = (d * d).astype(np.float32)
        rtop[:, blk, :] = _feat5_refs(b0 + np.arange(CAND))
    ghr = _bf16(
        np.concatenate([g.reshape(W, -1), rtop.reshape(5, -1)], axis=0)
    )

    # lhsT rows: 0..95 onehot(x), 96..100 yfeat; padded slots zero
    s2_lhsT = np.zeros((101, 2 * QHALF), np.float32)
    s2_lhsT[q_xs, np.arange(cnt_q)] = 1.0
    s2_lhsT[96:101, :cnt_q] = _feat5_queries(q_ys)

    maps = []
    for half in range(2):
        hs = slice(half * QHALF, (half + 1) * QHALF)
        maps.append({"lhsT": _bf16(s2_lhsT[:, hs]), "ghr": ghr})
    return maps


def _quantile95(vals):
    """torch.quantile / jnp.nanquantile 'linear' on finite values."""
    v = np.sort(np.asarray(vals, np.float64))
    n = v.size
    pos = 0.95 * (n - 1)
    lo = int(np.floor(pos))
    hi = min(lo + 1, n - 1)
    frac = pos - lo
    return v[lo] * (1.0 - frac) + v[hi] * frac


def _hd95_numpy_fallback(pred, true):
    """Pure-numpy path for data outside the compiled regime."""
    p_ys, p_xs = _side_points(pred)
    t_ys, t_xs = _side_points(true)
    if len(p_ys) == 0 or len(t_ys) == 0:
        return None
    pc = np.stack([p_ys, p_xs], -1).astype(np.float32)
    tc = np.stack([t_ys, t_xs], -1).astype(np.float32)
    vals = []
    for qc, rc in ((pc, tc), (tc, pc)):
        nbr = (len(rc) + BLK - 1) // BLK
        for jb in range(nbr):
            b = rc[jb * BLK : (jb + 1) * BLK]
            d2 = (
                (qc * qc).sum(-1)[:, None]
                + (b * b).sum(-1)[None, :]
                - 2.0 * (qc @ b.T)
            )
            vals.append(np.sqrt(np.maximum(d2.min(1), 0.0).astype(np.float32)))
    return _quantile95(np.concatenate(vals))


def _run_device(in_maps, trace=False):
    from concourse.bass_utils import run_bass_kernel_spmd

    nc = _get_nc()
    return run_bass_kernel_spmd(nc, in_maps, list(range(NCORES)), trace=trace)


def _decode_mins(raw):
    """[128, NTILES*NBLK] device layout -> [QHALF, NBLK] query-major d^2."""
    # column g*GRP*NBLK + k*NBLK + blk holds tile t = g*GRP+k; query
    # q = t*128 + partition
    return (
        raw.reshape(128, NTILES, NBLK).transpose(1, 0, 2).reshape(QHALF, NBLK)
    )


def kernel(input, target, _trace=False, _results_out=None):
    input = np.asarray(input)
    target = np.asarray(target)
    nimg = input.shape[0]

    # jobs: (image, direction). dir 0: queries=pred, refs=true (row mins);
    # dir 1: queries=true, refs=pred (col mins).
    jobs = []
    in_maps = []
    fallback = {}
    ok_mask = []
    for i in range(nimg):
        p_ys, p_xs = _side_points(input[i])
        t_ys, t_xs = _side_points(target[i])
        ok = len(p_ys) > 0 and len(t_ys) > 0
        ok_mask.append(ok)
        if not ok:
            continue
        built_row = _build_core_inputs(p_ys, p_xs, t_ys, t_xs)
        built_col = _build_core_inputs(t_ys, t_xs, p_ys, p_xs)
        if built_row is None or built_col is None or nimg != 2:
            fallback[i] = _hd95_numpy_fallback(input[i], target[i])
            continue
        jobs.append((i, 0, len(p_ys)))
        in_maps.extend(built_row)
        jobs.append((i, 1, len(t_ys)))
        in_maps.extend(built_col)

    hds = {}
    if jobs:
        while len(in_maps) < NCORES:  # pad to the full 8-core SPMD launch
            in_maps.append({k: v.copy() for k, v in in_maps[0].items()})
        res = _run_device(in_maps[:NCORES], trace=_trace)
        if _results_out is not None:
            _results_out.append(res)
        per_img_vals = {}
        for j, (img, _dir, cnt_q) in enumerate(jobs):
            d2 = np.concatenate(
                [
                    _decode_mins(res.results[2 * j]["mins"]),
                    _decode_mins(res.results[2 * j + 1]["mins"]),
                ]
            )[:cnt_q]
            assert d2.max() < 2.0 ** 25, "sentinel leaked into mins"
            dist = np.sqrt(d2.astype(np.float32))
            per_img_vals.setdefault(img, []).append(dist.ravel())
        for img, chunks in per_img_vals.items():
            hds[img] = _quantile95(np.concatenate(chunks))
    hds.update(fallback)

    n_ok = sum(ok_mask)
    if n_ok == 0:
        return np.float32(np.inf)
    total = sum(hds[i] for i in range(nimg) if ok_mask[i])
    return np.float32(total / n_ok)


# revision 6
# speedup vs baseline: 1.9037x; 1.0672x over previous
"""HD95 loss kernel for Trainium2 (Bass/Tile), 8 NeuronCores — banded gather.

Reference semantics: per image, threshold pred/true at 0.5, compact nonzero
pixel indices in row-major order, split each point list into blocks of 1000,
and for every (point, opposite-side block) pair take the min Euclidean
distance; the HD95 is the 95th linear-interpolation quantile over all finite
such mins (both directions), averaged over the batch.

Device algorithm (per image & direction, "queries" vs "ref blocks"):
separable squared-EDT with the row stage precomputed on the host, and the
column stage as a BANDED gather matmul. The host sorts each core's queries
by x and buckets them into NWIN x-windows of WSPAN columns, padding each
window to TPW tiles of 128 slots. Within a window the one-hot(x) only
needs WSPAN rows, so one bf16 matmul per tile with K = WSPAN+5:

  min d^2(q, blk) = min_c ( (y_q - (b0+c))^2 + g[x_q, c] )
  [onehot(x_q - 16w); y2h, y2l, y, 1, 1] @ [g[16w:16w+16] ; rtop]

then a DVE min-reduce over the 24 candidates of each block. The y-part is
bit-exact (exact hi/lo split of squares); g carries <=2^-9 relative bf16
rounding, far inside the 2e-2 harness gate. Query order is irrelevant: all
(query, block) mins are pooled into one quantile.

Core mapping: 8 cores = 4 (image x direction) jobs x 2 interleaved halves
of each x-window. Host does the O(N) compaction/sort/feature build and the
final O(50k) quantile; device does all O(K x window) distance work.
"""

import numpy as np

H = 96
W = 96
BLK = 1000        # reference cdist block size
NBLK = 5          # blocks per side (asserted from the data regime)
CAND = 24         # candidate image rows per block window (spans <= 23 here)
M = NBLK * CAND   # matmul free size (120 candidate columns)
NWIN = 6          # x-windows per core
WSPAN = 16        # image columns per window
TPW = 4           # query tiles of 128 per window (512 slots/window)
NTILES = NWIN * TPW           # 24 tiles per core
QHALF = NTILES * 128          # 3072 query slots per core
KB = WSPAN + 5                # matmul contraction (band + y features)
BIG = float(2 ** 26)  # sentinel (bf16-exact, >> max real d^2 of 18050)
NCORES = 8

_CACHE = {}


def _build_nc():
    import concourse.bacc as bacc
    import concourse.mybir as mybir
    import concourse.tile as tile

    f32 = mybir.dt.float32
    bf16 = mybir.dt.bfloat16
    nc = bacc.Bacc("TRN2", target_bir_lowering=False, debug=False)

    lhsT = nc.declare_dram_parameter("lhsT", [KB, QHALF], bf16, isOutput=False)
    # per-window rhs blocks side by side in the free dim: window w's rhs is
    # gg[:, w*M:(w+1)*M] = [g[16w:16w+16] ; rtop] (rtop repeated per window)
    gg = nc.declare_dram_parameter("gg", [KB, NWIN * M], bf16, isOutput=False)
    mins = nc.declare_dram_parameter(
        "mins", [128, NTILES * NBLK], f32, isOutput=True
    )

    X = mybir.AxisListType.X
    MIN = mybir.AluOpType.min
    NDMA = 4  # lhsT partition-row slices

    with tile.TileContext(nc) as tc:
        with (
            tc.tile_pool(name="const", bufs=1) as const,
            tc.tile_pool(name="ps", bufs=NWIN, space="PSUM") as psp,
        ):
            t_lhsT = const.tile([KB, QHALF], bf16)
            t_gg = const.tile([KB, NWIN * M], bf16)
            t_out = const.tile([128, NTILES * NBLK], f32)

            # rhs first (needed by every matmul), then lhsT in partition-row
            # slices (full 6KB lines) alternating across the two HWDGE
            # queues so issue cost parallelizes and each slice gets its own
            # ~15 GB/s DMA engine.
            nc.scalar.dma_start(t_gg[:], gg[:])
            bounds = np.linspace(0, KB, NDMA + 1).astype(int)
            for i in range(NDMA):
                sl = slice(int(bounds[i]), int(bounds[i + 1]))
                eng = nc.sync if i % 2 == 0 else nc.scalar
                eng.dma_start(t_lhsT[sl, :], lhsT[sl, :])

            for w in range(NWIN):
                ps = psp.tile([128, TPW, NBLK, CAND], f32, tag="ps")
                for k in range(TPW):
                    t = w * TPW + k
                    nc.tensor.matmul(
                        ps[:, k, :, :],
                        t_lhsT[:, t * 128 : (t + 1) * 128],
                        t_gg[:, w * M : (w + 1) * M],
                        start=True,
                        stop=True,
                    )
                o0 = w * TPW * NBLK
                nc.vector.tensor_reduce(
                    t_out[:, o0 : o0 + TPW * NBLK],
                    ps[:, :, :, :],
                    axis=X,
                    op=MIN,
                )
                nc.sync.dma_start(
                    mins[:, o0 : o0 + TPW * NBLK], t_out[:, o0 : o0 + TPW * NBLK]
                )

    nc.compile()
    return nc


def _get_nc():
    if "nc" not in _CACHE:
        _CACHE["nc"] = _build_nc()
    return _CACHE["nc"]


def _bf16(a):
    from ml_dtypes import bfloat16

    return np.asarray(a, np.float32).astype(bfloat16)


def _hilo(v):
    """Split integer-valued array into (multiple-of-128, remainder<128)."""
    v = np.asarray(v, np.float64)
    lo = np.mod(v, 128.0)
    return (v - lo).astype(np.float32), lo.astype(np.float32)


def _side_points(img):
    """Compacted nonzero pixel coords, row-major ascending (matches
    jnp.nonzero order)."""
    m = (np.asarray(img) > 0.5).reshape(-1)
    idx = np.nonzero(m)[0]
    ys = (idx // W).astype(np.int64)
    xs = (idx % W).astype(np.int64)
    return ys, xs


def _feat5_queries(vals):
    """[v2h, v2l, v, 1, 1] feature rows for the squared-term side."""
    v = np.asarray(vals, np.float64)
    h, l = _hilo(v * v)
    one = np.ones_like(v, np.float32)
    return np.stack([h, l, v.astype(np.float32), one, one])


def _feat5_refs(vals):
    """[1, 1, -2v, v2h, v2l] feature rows for the reference side."""
    v = np.asarray(vals, np.float64)
    h, l = _hilo(v * v)
    one = np.ones_like(v, np.float32)
    return np.stack([one, one, (-2.0 * v).astype(np.float32), h, l])


def _build_g_rtop(r_ys, r_xs, cnt_r):
    """g[x, blk, cand] table (f32, BIG sentinel) + rtop features, or None
    if outside the compiled regime."""
    xgrid = np.arange(W, dtype=np.float64)
    g = np.full((W, NBLK, CAND), BIG, np.float32)
    rtop = np.empty((5, NBLK, CAND), np.float32)
    for blk in range(NBLK):
        lo, hi = blk * BLK, min((blk + 1) * BLK, cnt_r)
        ys_b, xs_b = r_ys[lo:hi], r_xs[lo:hi]
        b0 = int(ys_b[0])
        if int(ys_b[-1]) - b0 + 1 > CAND:
            return None
        for c in np.unique(ys_b - b0):
            xs_c = xs_b[ys_b - b0 == c].astype(np.float64)
            d = np.abs(xgrid[:, None] - xs_c[None, :]).min(1)
            g[:, blk, c] = (d * d).astype(np.float32)
        rtop[:, blk, :] = _feat5_refs(b0 + np.arange(CAND))
    return g, rtop


def _build_core_inputs(q_ys, q_xs, r_ys, r_xs):
    """Host-side feature build for one (image, direction) job.

    Returns (two per-core input maps, two per-core valid-slot masks), or
    None if the data falls outside the compiled regime.
    """
    cnt_q, cnt_r = len(q_ys), len(r_ys)
    if not (0 < cnt_q and 0 < cnt_r <= NBLK * BLK):
        return None
    if (cnt_r + BLK - 1) // BLK != NBLK:
        return None

    built = _build_g_rtop(r_ys, r_xs, cnt_r)
    if built is None:
        return None
    g, rtop = built

    # gg: per-window rhs blocks [KB, NWIN*M]
    ggm = np.empty((KB, NWIN, M), np.float32)
    for w in range(NWIN):
        ggm[:WSPAN, w] = g[w * WSPAN : (w + 1) * WSPAN].reshape(WSPAN, M)
        ggm[WSPAN:, w] = rtop.reshape(5, M)
    gg = _bf16(ggm.reshape(KB, NWIN * M))

    # bucket queries by x-window, split each window across the two cores
    lhsT = np.zeros((2, KB, QHALF), np.float32)
    valid = np.zeros((2, QHALF), bool)
    win = q_xs // WSPAN
    for w in range(NWIN):
        (idx_w,) = np.nonzero(win == w)
        for half in range(2):
            part = idx_w[half::2]
            if len(part) > TPW * 128:
                return None
            s0 = w * TPW * 128
            sl = slice(s0, s0 + len(part))
            lhsT[half, q_xs[part] - w * WSPAN, np.arange(sl.start, sl.stop)] = 1.0
            lhsT[half, WSPAN:, sl] = _feat5_queries(q_ys[part])
            valid[half, sl] = True

    maps = [{"lhsT": _bf16(lhsT[h]), "gg": gg} for h in range(2)]
    return maps, [valid[0], valid[1]]


def _quantile95(vals):
    """torch.quantile / jnp.nanquantile 'linear' on finite values."""
    v = np.sort(np.asarray(vals, np.float64))
    n = v.size
    pos = 0.95 * (n - 1)
    lo = int(np.floor(pos))
    hi = min(lo + 1, n - 1)
    frac = pos - lo
    return v[lo] * (1.0 - frac) + v[hi] * frac


def _hd95_numpy_fallback(pred, true):
    """Pure-numpy path for data outside the compiled regime."""
    p_ys, p_xs = _side_points(pred)
    t_ys, t_xs = _side_points(true)
    if len(p_ys) == 0 or len(t_ys) == 0:
        return None
    pc = np.stack([p_ys, p_xs], -1).astype(np.float32)
    tc = np.stack([t_ys, t_xs], -1).astype(np.float32)
    vals = []
    for qc, rc in ((pc, tc), (tc, pc)):
        nbr = (len(rc) + BLK - 1) // BLK
        for jb in range(nbr):
            b = rc[jb * BLK : (jb + 1) * BLK]
            d2 = (
                (qc * qc).sum(-1)[:, None]
                + (b * b).sum(-1)[None, :]
                - 2.0 * (qc @ b.T)
            )
            vals.append(np.sqrt(np.maximum(d2.min(1), 0.0).astype(np.float32)))
    return _quantile95(np.concatenate(vals))


def _run_device(in_maps, trace=False):
    from concourse.bass_utils import run_bass_kernel_spmd

    nc = _get_nc()
    return run_bass_kernel_spmd(nc, in_maps, list(range(NCORES)), trace=trace)


def _decode_mins(raw):
    """[128, NTILES*NBLK] device layout -> [QHALF, NBLK] slot-major d^2."""
    return (
        raw.reshape(128, NTILES, NBLK).transpose(1, 0, 2).reshape(QHALF, NBLK)
    )


def kernel(input, target, _trace=False, _results_out=None):
    input = np.asarray(input)
    target = np.asarray(target)
    nimg = input.shape[0]

    jobs = []
    in_maps = []
    valid_masks = []
    fallback = {}
    ok_mask = []
    for i in range(nimg):
        p_ys, p_xs = _side_points(input[i])
        t_ys, t_xs = _side_points(target[i])
        ok = len(p_ys) > 0 and len(t_ys) > 0
        ok_mask.append(ok)
        if not ok:
            continue
        built_row = _build_core_inputs(p_ys, p_xs, t_ys, t_xs)
        built_col = _build_core_inputs(t_ys, t_xs, p_ys, p_xs)
        if built_row is None or built_col is None or nimg != 2:
            fallback[i] = _hd95_numpy_fallback(input[i], target[i])
            continue
        jobs.append((i, 0))
        in_maps.extend(built_row[0])
        valid_masks.extend(built_row[1])
        jobs.append((i, 1))
        in_maps.extend(built_col[0])
        valid_masks.extend(built_col[1])

    hds = {}
    if jobs:
        while len(in_maps) < NCORES:  # pad to the full 8-core SPMD launch
            in_maps.append({k: v.copy() for k, v in in_maps[0].items()})
        res = _run_device(in_maps[:NCORES], trace=_trace)
        if _results_out is not None:
            _results_out.append(res)
        per_img_vals = {}
        for j, (img, _dir) in enumerate(jobs):
            d2 = np.concatenate(
                [
                    _decode_mins(res.results[2 * j]["mins"])[valid_masks[2 * j]],
                    _decode_mins(res.results[2 * j + 1]["mins"])[
                        valid_masks[2 * j + 1]
                    ],
                ]
            )
            assert d2.max() < 2.0 ** 25, "sentinel leaked into mins"
            dist = np.sqrt(d2.astype(np.float32))
            per_img_vals.setdefault(img, []).append(dist.ravel())
        for img, chunks in per_img_vals.items():
            hds[img] = _quantile95(np.concatenate(chunks))
    hds.update(fallback)

    n_ok = sum(ok_mask)
    if n_ok == 0:
        return np.float32(np.inf)
    total = sum(hds[i] for i in range(nimg) if ok_mask[i])
    return np.float32(total / n_ok)


# revision 7
# speedup vs baseline: 2.0022x; 1.0517x over previous
"""HD95 loss kernel for Trainium2 (Bass/Tile), 8 NeuronCores — banded gather.

Reference semantics: per image, threshold pred/true at 0.5, compact nonzero
pixel indices in row-major order, split each point list into blocks of 1000,
and for every (point, opposite-side block) pair take the min Euclidean
distance; the HD95 is the 95th linear-interpolation quantile over all finite
such mins (both directions), averaged over the batch.

Device algorithm (per image & direction, "queries" vs "ref blocks"):
separable squared-EDT with the row stage precomputed on the host, and the
column stage as a BANDED gather matmul. The host sorts each core's queries
by x and buckets them into NWIN x-windows of WSPAN columns, padding each
window to TPW tiles of 128 slots. Within a window the one-hot(x) only
needs WSPAN rows, so one bf16 matmul per tile with K = WSPAN+5:

  min d^2(q, blk) = min_c ( (y_q - (b0+c))^2 + g[x_q, c] )
  [onehot(x_q - 16w); y2h, y2l, y, 1, 1] @ [g[16w:16w+16] ; rtop]

then a DVE min-reduce over the 24 candidates of each block. The y-part is
bit-exact (exact hi/lo split of squares); g carries <=2^-9 relative bf16
rounding, far inside the 2e-2 harness gate. Query order is irrelevant: all
(query, block) mins are pooled into one quantile.

Core mapping: 8 cores = 4 (image x direction) jobs x 2 interleaved halves
of each x-window. Host does the O(N) compaction/sort/feature build and the
final O(50k) quantile; device does all O(K x window) distance work.
"""

import numpy as np

H = 96
W = 96
BLK = 1000        # reference cdist block size
NBLK = 5          # blocks per side (asserted from the data regime)
CAND = 24         # candidate image rows per block window (spans <= 23 here)
M = NBLK * CAND   # matmul free size (120 candidate columns)
NWIN = 8          # x-windows per core
WSPAN = 12        # image columns per window
TPW = 3           # query tiles of 128 per window (384 slots/window)
NTILES = NWIN * TPW           # 24 tiles per core
QHALF = NTILES * 128          # 3072 query slots per core
KB = WSPAN + 5                # matmul contraction (band + y features)
BIG = float(2 ** 26)  # sentinel (bf16-exact, >> max real d^2 of 18050)
NCORES = 8

_CACHE = {}


def _build_nc():
    import concourse.bacc as bacc
    import concourse.mybir as mybir
    import concourse.tile as tile

    f32 = mybir.dt.float32
    bf16 = mybir.dt.bfloat16
    nc = bacc.Bacc("TRN2", target_bir_lowering=False, debug=False)

    # one concatenated input: [gg (per-window rhs blocks) | lhsT]; window
    # w's rhs is cat[:, w*M:(w+1)*M] = [g[12w:12w+12] ; rtop], the banded
    # one-hot lhsT starts at column NWIN*M
    cat = nc.declare_dram_parameter(
        "cat", [KB, NWIN * M + QHALF], bf16, isOutput=False
    )
    mins = nc.declare_dram_parameter(
        "mins", [128, NTILES * NBLK], f32, isOutput=True
    )

    X = mybir.AxisListType.X
    MIN = mybir.AluOpType.min
    NDMA = 4  # cat partition-row slices
    NBANK = NTILES // 4  # psum banks, 4 query tiles each (bank-aligned)

    with tile.TileContext(nc) as tc:
        with (
            tc.tile_pool(name="const", bufs=1) as const,
            tc.tile_pool(name="ps", bufs=NBANK, space="PSUM") as psp,
        ):
            t_cat = const.tile([KB, NWIN * M + QHALF], bf16)
            t_out = const.tile([128, NTILES * NBLK], f32)

            # the whole input in partition-row slices (full ~8KB lines)
            # alternating across the two HWDGE queues so issue cost
            # parallelizes and each slice gets its own ~15 GB/s DMA engine
            bounds = np.linspace(0, KB, NDMA + 1).astype(int)
            for i in range(NDMA):
                sl = slice(int(bounds[i]), int(bounds[i + 1]))
                eng = nc.sync if i % 2 == 0 else nc.scalar
                eng.dma_start(t_cat[sl, :], cat[sl, :])

            L0 = NWIN * M  # lhsT column offset within cat
            for b in range(NBANK):
                ps = psp.tile([128, 4, NBLK, CAND], f32, tag="ps")
                for k in range(4):
                    t = b * 4 + k
                    w = t // TPW
                    nc.tensor.matmul(
                        ps[:, k, :, :],
                        t_cat[:, L0 + t * 128 : L0 + (t + 1) * 128],
                        t_cat[:, w * M : (w + 1) * M],
                        start=True,
                        stop=True,
                    )
                o0 = b * 4 * NBLK
                nc.vector.tensor_reduce(
                    t_out[:, o0 : o0 + 4 * NBLK],
                    ps[:, :, :, :],
                    axis=X,
                    op=MIN,
                )
                nc.sync.dma_start(
                    mins[:, o0 : o0 + 4 * NBLK], t_out[:, o0 : o0 + 4 * NBLK]
                )

    nc.compile()
    return nc


def _get_nc():
    if "nc" not in _CACHE:
        _CACHE["nc"] = _build_nc()
    return _CACHE["nc"]


def _bf16(a):
    from ml_dtypes import bfloat16

    return np.asarray(a, np.float32).astype(bfloat16)


def _hilo(v):
    """Split integer-valued array into (multiple-of-128, remainder<128)."""
    v = np.asarray(v, np.float64)
    lo = np.mod(v, 128.0)
    return (v - lo).astype(np.float32), lo.astype(np.float32)


def _side_points(img):
    """Compacted nonzero pixel coords, row-major ascending (matches
    jnp.nonzero order)."""
    m = (np.asarray(img) > 0.5).reshape(-1)
    idx = np.nonzero(m)[0]
    ys = (idx // W).astype(np.int64)
    xs = (idx % W).astype(np.int64)
    return ys, xs


def _feat5_queries(vals):
    """[v2h, v2l, v, 1, 1] feature rows for the squared-term side."""
    v = np.asarray(vals, np.float64)
    h, l = _hilo(v * v)
    one = np.ones_like(v, np.float32)
    return np.stack([h, l, v.astype(np.float32), one, one])


def _feat5_refs(vals):
    """[1, 1, -2v, v2h, v2l] feature rows for the reference side."""
    v = np.asarray(vals, np.float64)
    h, l = _hilo(v * v)
    one = np.ones_like(v, np.float32)
    return np.stack([one, one, (-2.0 * v).astype(np.float32), h, l])


def _build_g_rtop(r_ys, r_xs, cnt_r):
    """g[x, blk, cand] table (f32, BIG sentinel) + rtop features, or None
    if outside the compiled regime."""
    xgrid = np.arange(W, dtype=np.float64)
    g = np.full((W, NBLK, CAND), BIG, np.float32)
    rtop = np.empty((5, NBLK, CAND), np.float32)
    for blk in range(NBLK):
        lo, hi = blk * BLK, min((blk + 1) * BLK, cnt_r)
        ys_b, xs_b = r_ys[lo:hi], r_xs[lo:hi]
        b0 = int(ys_b[0])
        if int(ys_b[-1]) - b0 + 1 > CAND:
            return None
        for c in np.unique(ys_b - b0):
            xs_c = xs_b[ys_b - b0 == c].astype(np.float64)
            d = np.abs(xgrid[:, None] - xs_c[None, :]).min(1)
            g[:, blk, c] = (d * d).astype(np.float32)
        rtop[:, blk, :] = _feat5_refs(b0 + np.arange(CAND))
    return g, rtop


def _build_core_inputs(q_ys, q_xs, r_ys, r_xs):
    """Host-side feature build for one (image, direction) job.

    Returns (two per-core input maps, two per-core valid-slot masks), or
    None if the data falls outside the compiled regime.
    """
    cnt_q, cnt_r = len(q_ys), len(r_ys)
    if not (0 < cnt_q and 0 < cnt_r <= NBLK * BLK):
        return None
    if (cnt_r + BLK - 1) // BLK != NBLK:
        return None

    built = _build_g_rtop(r_ys, r_xs, cnt_r)
    if built is None:
        return None
    g, rtop = built

    # gg: per-window rhs blocks [KB, NWIN*M]
    ggm = np.empty((KB, NWIN, M), np.float32)
    for w in range(NWIN):
        ggm[:WSPAN, w] = g[w * WSPAN : (w + 1) * WSPAN].reshape(WSPAN, M)
        ggm[WSPAN:, w] = rtop.reshape(5, M)
    gg = ggm.reshape(KB, NWIN * M)

    # bucket queries by x-window, split each window across the two cores
    lhsT = np.zeros((2, KB, QHALF), np.float32)
    valid = np.zeros((2, QHALF), bool)
    win = q_xs // WSPAN
    for w in range(NWIN):
        (idx_w,) = np.nonzero(win == w)
        for half in range(2):
            part = idx_w[half::2]
            if len(part) > TPW * 128:
                return None
            s0 = w * TPW * 128
            sl = slice(s0, s0 + len(part))
            lhsT[half, q_xs[part] - w * WSPAN, np.arange(sl.start, sl.stop)] = 1.0
            lhsT[half, WSPAN:, sl] = _feat5_queries(q_ys[part])
            valid[half, sl] = True

    maps = [
        {"cat": _bf16(np.concatenate([gg, lhsT[h]], axis=1))} for h in range(2)
    ]
    return maps, [valid[0], valid[1]]


def _quantile95(vals):
    """torch.quantile / jnp.nanquantile 'linear' on finite values."""
    v = np.sort(np.asarray(vals, np.float64))
    n = v.size
    pos = 0.95 * (n - 1)
    lo = int(np.floor(pos))
    hi = min(lo + 1, n - 1)
    frac = pos - lo
    return v[lo] * (1.0 - frac) + v[hi] * frac


def _hd95_numpy_fallback(pred, true):
    """Pure-numpy path for data outside the compiled regime."""
    p_ys, p_xs = _side_points(pred)
    t_ys, t_xs = _side_points(true)
    if len(p_ys) == 0 or len(t_ys) == 0:
        return None
    pc = np.stack([p_ys, p_xs], -1).astype(np.float32)
    tc = np.stack([t_ys, t_xs], -1).astype(np.float32)
    vals = []
    for qc, rc in ((pc, tc), (tc, pc)):
        nbr = (len(rc) + BLK - 1) // BLK
        for jb in range(nbr):
            b = rc[jb * BLK : (jb + 1) * BLK]
            d2 = (
                (qc * qc).sum(-1)[:, None]
                + (b * b).sum(-1)[None, :]
                - 2.0 * (qc @ b.T)
            )
            vals.append(np.sqrt(np.maximum(d2.min(1), 0.0).astype(np.float32)))
    return _quantile95(np.concatenate(vals))


def _run_device(in_maps, trace=False):
    from concourse.bass_utils import run_bass_kernel_spmd

    nc = _get_nc()
    return run_bass_kernel_spmd(nc, in_maps, list(range(NCORES)), trace=trace)


def _decode_mins(raw):
    """[128, NTILES*NBLK] device layout -> [QHALF, NBLK] slot-major d^2."""
    return (
        raw.reshape(128, NTILES, NBLK).transpose(1, 0, 2).reshape(QHALF, NBLK)
    )


def kernel(input, target, _trace=False, _results_out=None):
    input = np.asarray(input)
    target = np.asarray(target)
    nimg = input.shape[0]

    jobs = []
    in_maps = []
    valid_masks = []
    fallback = {}
    ok_mask = []
    for i in range(nimg):
        p_ys, p_xs = _side_points(input[i])
        t_ys, t_xs = _side_points(target[i])
        ok = len(p_ys) > 0 and len(t_ys) > 0
        ok_mask.append(ok)
        if not ok:
            continue
        built_row = _build_core_inputs(p_ys, p_xs, t_ys, t_xs)
        built_col = _build_core_inputs(t_ys, t_xs, p_ys, p_xs)
        if built_row is None or built_col is None or nimg != 2:
            fallback[i] = _hd95_numpy_fallback(input[i], target[i])
            continue
        jobs.append((i, 0))
        in_maps.extend(built_row[0])
        valid_masks.extend(built_row[1])
        jobs.append((i, 1))
        in_maps.extend(built_col[0])
        valid_masks.extend(built_col[1])

    hds = {}
    if jobs:
        while len(in_maps) < NCORES:  # pad to the full 8-core SPMD launch
            in_maps.append({k: v.copy() for k, v in in_maps[0].items()})
        res = _run_device(in_maps[:NCORES], trace=_trace)
        if _results_out is not None:
            _results_out.append(res)
        per_img_vals = {}
        for j, (img, _dir) in enumerate(jobs):
            d2 = np.concatenate(
                [
                    _decode_mins(res.results[2 * j]["mins"])[valid_masks[2 * j]],
                    _decode_mins(res.results[2 * j + 1]["mins"])[
                        valid_masks[2 * j + 1]
                    ],
                ]
            )
            assert d2.max() < 2.0 ** 25, "sentinel leaked into mins"
            dist = np.sqrt(d2.astype(np.float32))
            per_img_vals.setdefault(img, []).append(dist.ravel())
        for img, chunks in per_img_vals.items():
            hds[img] = _quantile95(np.concatenate(chunks))
    hds.update(fallback)

    n_ok = sum(ok_mask)
    if n_ok == 0:
        return np.float32(np.inf)
    total = sum(hds[i] for i in range(nimg) if ok_mask[i])
    return np.float32(total / n_ok)


# revision 8
# speedup vs baseline: 2.2187x; 1.1081x over previous
"""HD95 loss kernel for Trainium2 (Bass/Tile), 8 NeuronCores — banded gather.

Reference semantics: per image, threshold pred/true at 0.5, compact nonzero
pixel indices in row-major order, split each point list into blocks of 1000,
and for every (point, opposite-side block) pair take the min Euclidean
distance; the HD95 is the 95th linear-interpolation quantile over all finite
such mins (both directions), averaged over the batch.

Device algorithm (per image & direction, "queries" vs "ref blocks"):
separable squared-EDT with the row stage precomputed on the host, and the
column stage as a BANDED gather matmul. The host sorts each core's queries
by x and buckets them into NWIN x-windows of WSPAN columns, padding each
window to TPW tiles of 128 slots. Within a window the one-hot(x) only
needs WSPAN rows, so one bf16 matmul per tile with K = WSPAN+5:

  min d^2(q, blk) = min_c ( (y_q - (b0+c))^2 + g[x_q, c] )
  [onehot(x_q - 16w); y2h, y2l, y, 1, 1] @ [g[16w:16w+16] ; rtop]

then a DVE min-reduce over the 24 candidates of each block. The y-part is
bit-exact (exact hi/lo split of squares); g carries <=2^-9 relative bf16
rounding, far inside the 2e-2 harness gate. Query order is irrelevant: all
(query, block) mins are pooled into one quantile.

Core mapping: 8 cores = 4 (image x direction) jobs x 2 interleaved halves
of each x-window. Host does the O(N) compaction/sort/feature build and the
final O(50k) quantile; device does all O(K x window) distance work.
"""

import numpy as np

H = 96
W = 96
BLK = 1000        # reference cdist block size
NBLK = 5          # blocks per side (asserted from the data regime)
CAND = 24         # candidate image rows per block window (spans <= 23 here)
M = NBLK * CAND   # matmul free size (120 candidate columns)
NWIN = 8          # x-windows per core
WSPAN = 12        # image columns per window
TPW = 3           # query tiles of 128 per window (384 slots/window)
NTILES = NWIN * TPW           # 24 tiles per core
QHALF = NTILES * 128          # 3072 query slots per core
KB = WSPAN + 5                # matmul contraction (band + y features)
BIG = float(2 ** 26)  # sentinel (bf16-exact, >> max real d^2 of 18050)
NCORES = 8

_CACHE = {}


def _build_nc():
    import concourse.bacc as bacc
    import concourse.mybir as mybir
    import concourse.tile as tile

    f32 = mybir.dt.float32
    bf16 = mybir.dt.bfloat16
    nc = bacc.Bacc("TRN2", target_bir_lowering=False, debug=False)

    # one concatenated input: [gg (per-window rhs blocks) | lhsT]; window
    # w's rhs is cat[:, w*M:(w+1)*M] = [g[12w:12w+12] ; rtop], the banded
    # one-hot lhsT starts at column NWIN*M
    cat = nc.declare_dram_parameter(
        "cat", [KB, NWIN * M + QHALF], bf16, isOutput=False
    )
    mins = nc.declare_dram_parameter(
        "mins", [128, NTILES * NBLK], f32, isOutput=True
    )

    X = mybir.AxisListType.X
    MIN = mybir.AluOpType.min
    NDMA = 4  # cat partition-row slices
    NBANK = NTILES // 4  # psum banks, 4 query tiles each (bank-aligned)

    with tile.TileContext(nc) as tc:
        with (
            tc.tile_pool(name="const", bufs=1) as const,
            tc.tile_pool(name="ps", bufs=NBANK, space="PSUM") as psp,
        ):
            t_cat = const.tile([KB, NWIN * M + QHALF], bf16)
            t_out = const.tile([128, NTILES * NBLK], f32)

            # quadrant-sliced input DMA (2 row-groups x 2 column-halves,
            # ~4KB lines): the first column half carries gg plus the first
            # two psum banks' query tiles, so their matmuls start while
            # the second half is still streaming. Alternate queues so the
            # ~720ns per-dma_start issue cost parallelizes.
            NCAT = NWIN * M + QHALF
            csplit = NWIN * M + 8 * 128  # gg + tiles 0..7 (banks 0-1)
            cbounds = [0, csplit, NCAT]
            rbounds = [0, KB // 2, KB]
            di = 0
            for c in range(2):
                for r in range(2):
                    rs = slice(rbounds[r], rbounds[r + 1])
                    cs = slice(cbounds[c], cbounds[c + 1])
                    eng = nc.sync if di % 2 == 0 else nc.scalar
                    eng.dma_start(t_cat[rs, cs], cat[rs, cs])
                    di += 1

            L0 = NWIN * M  # lhsT column offset within cat
            for b in range(NBANK):
                ps = psp.tile([128, 4, NBLK, CAND], f32, tag="ps")
                for k in range(4):
                    t = b * 4 + k
                    w = t // TPW
                    nc.tensor.matmul(
                        ps[:, k, :, :],
                        t_cat[:, L0 + t * 128 : L0 + (t + 1) * 128],
                        t_cat[:, w * M : (w + 1) * M],
                        start=True,
                        stop=True,
                    )
                o0 = b * 4 * NBLK
                nc.vector.tensor_reduce(
                    t_out[:, o0 : o0 + 4 * NBLK],
                    ps[:, :, :, :],
                    axis=X,
                    op=MIN,
                )
                nc.sync.dma_start(
                    mins[:, o0 : o0 + 4 * NBLK], t_out[:, o0 : o0 + 4 * NBLK]
                )

    nc.compile()
    return nc


def _get_nc():
    if "nc" not in _CACHE:
        _CACHE["nc"] = _build_nc()
    return _CACHE["nc"]


def _bf16(a):
    from ml_dtypes import bfloat16

    return np.asarray(a, np.float32).astype(bfloat16)


def _hilo(v):
    """Split integer-valued array into (multiple-of-128, remainder<128)."""
    v = np.asarray(v, np.float64)
    lo = np.mod(v, 128.0)
    return (v - lo).astype(np.float32), lo.astype(np.float32)


def _side_points(img):
    """Compacted nonzero pixel coords, row-major ascending (matches
    jnp.nonzero order)."""
    m = (np.asarray(img) > 0.5).reshape(-1)
    idx = np.nonzero(m)[0]
    ys = (idx // W).astype(np.int64)
    xs = (idx % W).astype(np.int64)
    return ys, xs


def _feat5_queries(vals):
    """[v2h, v2l, v, 1, 1] feature rows for the squared-term side."""
    v = np.asarray(vals, np.float64)
    h, l = _hilo(v * v)
    one = np.ones_like(v, np.float32)
    return np.stack([h, l, v.astype(np.float32), one, one])


def _feat5_refs(vals):
    """[1, 1, -2v, v2h, v2l] feature rows for the reference side."""
    v = np.asarray(vals, np.float64)
    h, l = _hilo(v * v)
    one = np.ones_like(v, np.float32)
    return np.stack([one, one, (-2.0 * v).astype(np.float32), h, l])


def _build_g_rtop(r_ys, r_xs, cnt_r):
    """g[x, blk, cand] table (f32, BIG sentinel) + rtop features, or None
    if outside the compiled regime."""
    xgrid = np.arange(W, dtype=np.float64)
    g = np.full((W, NBLK, CAND), BIG, np.float32)
    rtop = np.empty((5, NBLK, CAND), np.float32)
    for blk in range(NBLK):
        lo, hi = blk * BLK, min((blk + 1) * BLK, cnt_r)
        ys_b, xs_b = r_ys[lo:hi], r_xs[lo:hi]
        b0 = int(ys_b[0])
        if int(ys_b[-1]) - b0 + 1 > CAND:
            return None
        for c in np.unique(ys_b - b0):
            xs_c = xs_b[ys_b - b0 == c].astype(np.float64)
            d = np.abs(xgrid[:, None] - xs_c[None, :]).min(1)
            g[:, blk, c] = (d * d).astype(np.float32)
        rtop[:, blk, :] = _feat5_refs(b0 + np.arange(CAND))
    return g, rtop


def _build_core_inputs(q_ys, q_xs, r_ys, r_xs):
    """Host-side feature build for one (image, direction) job.

    Returns (two per-core input maps, two per-core valid-slot masks), or
    None if the data falls outside the compiled regime.
    """
    cnt_q, cnt_r = len(q_ys), len(r_ys)
    if not (0 < cnt_q and 0 < cnt_r <= NBLK * BLK):
        return None
    if (cnt_r + BLK - 1) // BLK != NBLK:
        return None

    built = _build_g_rtop(r_ys, r_xs, cnt_r)
    if built is None:
        return None
    g, rtop = built

    # gg: per-window rhs blocks [KB, NWIN*M]
    ggm = np.empty((KB, NWIN, M), np.float32)
    for w in range(NWIN):
        ggm[:WSPAN, w] = g[w * WSPAN : (w + 1) * WSPAN].reshape(WSPAN, M)
        ggm[WSPAN:, w] = rtop.reshape(5, M)
    gg = ggm.reshape(KB, NWIN * M)

    # bucket queries by x-window, split each window across the two cores
    lhsT = np.zeros((2, KB, QHALF), np.float32)
    valid = np.zeros((2, QHALF), bool)
    win = q_xs // WSPAN
    for w in range(NWIN):
        (idx_w,) = np.nonzero(win == w)
        for half in range(2):
            part = idx_w[half::2]
            if len(part) > TPW * 128:
                return None
            s0 = w * TPW * 128
            sl = slice(s0, s0 + len(part))
            lhsT[half, q_xs[part] - w * WSPAN, np.arange(sl.start, sl.stop)] = 1.0
            lhsT[half, WSPAN:, sl] = _feat5_queries(q_ys[part])
            valid[half, sl] = True

    maps = [
        {"cat": _bf16(np.concatenate([gg, lhsT[h]], axis=1))} for h in range(2)
    ]
    return maps, [valid[0], valid[1]]


def _quantile95(vals):
    """torch.quantile / jnp.nanquantile 'linear' on finite values."""
    v = np.sort(np.asarray(vals, np.float64))
    n = v.size
    pos = 0.95 * (n - 1)
    lo = int(np.floor(pos))
    hi = min(lo + 1, n - 1)
    frac = pos - lo
    return v[lo] * (1.0 - frac) + v[hi] * frac


def _hd95_numpy_fallback(pred, true):
    """Pure-numpy path for data outside the compiled regime."""
    p_ys, p_xs = _side_points(pred)
    t_ys, t_xs = _side_points(true)
    if len(p_ys) == 0 or len(t_ys) == 0:
        return None
    pc = np.stack([p_ys, p_xs], -1).astype(np.float32)
    tc = np.stack([t_ys, t_xs], -1).astype(np.float32)
    vals = []
    for qc, rc in ((pc, tc), (tc, pc)):
        nbr = (len(rc) + BLK - 1) // BLK
        for jb in range(nbr):
            b = rc[jb * BLK : (jb + 1) * BLK]
            d2 = (
                (qc * qc).sum(-1)[:, None]
                + (b * b).sum(-1)[None, :]
                - 2.0 * (qc @ b.T)
            )
            vals.append(np.sqrt(np.maximum(d2.min(1), 0.0).astype(np.float32)))
    return _quantile95(np.concatenate(vals))


def _run_device(in_maps, trace=False):
    from concourse.bass_utils import run_bass_kernel_spmd

    nc = _get_nc()
    return run_bass_kernel_spmd(nc, in_maps, list(range(NCORES)), trace=trace)


def _decode_mins(raw):
    """[128, NTILES*NBLK] device layout -> [QHALF, NBLK] slot-major d^2."""
    return (
        raw.reshape(128, NTILES, NBLK).transpose(1, 0, 2).reshape(QHALF, NBLK)
    )


def kernel(input, target, _trace=False, _results_out=None):
    input = np.asarray(input)
    target = np.asarray(target)
    nimg = input.shape[0]

    jobs = []
    in_maps = []
    valid_masks = []
    fallback = {}
    ok_mask = []
    for i in range(nimg):
        p_ys, p_xs = _side_points(input[i])
        t_ys, t_xs = _side_points(target[i])
        ok = len(p_ys) > 0 and len(t_ys) > 0
        ok_mask.append(ok)
        if not ok:
            continue
        built_row = _build_core_inputs(p_ys, p_xs, t_ys, t_xs)
        built_col = _build_core_inputs(t_ys, t_xs, p_ys, p_xs)
        if built_row is None or built_col is None or nimg != 2:
            fallback[i] = _hd95_numpy_fallback(input[i], target[i])
            continue
        jobs.append((i, 0))
        in_maps.extend(built_row[0])
        valid_masks.extend(built_row[1])
        jobs.append((i, 1))
        in_maps.extend(built_col[0])
        valid_masks.extend(built_col[1])

    hds = {}
    if jobs:
        while len(in_maps) < NCORES:  # pad to the full 8-core SPMD launch
            in_maps.append({k: v.copy() for k, v in in_maps[0].items()})
        res = _run_device(in_maps[:NCORES], trace=_trace)
        if _results_out is not None:
            _results_out.append(res)
        per_img_vals = {}
        for j, (img, _dir) in enumerate(jobs):
            d2 = np.concatenate(
                [
                    _decode_mins(res.results[2 * j]["mins"])[valid_masks[2 * j]],
                    _decode_mins(res.results[2 * j + 1]["mins"])[
                        valid_masks[2 * j + 1]
                    ],
                ]
            )
            assert d2.max() < 2.0 ** 25, "sentinel leaked into mins"
            dist = np.sqrt(d2.astype(np.float32))
            per_img_vals.setdefault(img, []).append(dist.ravel())
        for img, chunks in per_img_vals.items():
            hds[img] = _quantile95(np.concatenate(chunks))
    hds.update(fallback)

    n_ok = sum(ok_mask)
    if n_ok == 0:
        return np.float32(np.inf)
    total = sum(hds[i] for i in range(nimg) if ok_mask[i])
    return np.float32(total / n_ok)
